# revision 30
# baseline (speedup 1.0000x reference)
"""Multi-head attention Bass/Tile kernel for TRN2, 8-core SPMD.

Sharding: core c handles batch b = c//2, query-half qh = c%2. The host
rotates the token axis per core so query rows sit at [0:TQ] (attention is
key-permutation invariant), and gathers the unmasked keys (mask compaction)
so K/V projection + attention only touch TK <= T key tokens.

Architecture (measured ~257us vs 378us baseline):
  - Software-pipelined emission: Tile's scheduler is an out-of-order
    per-engine list scheduler (priority = emission order), so Q/K/V
    projections are emitted per head-pair interleaved with the attention
    loop; attention itself is emitted under tc.high_priority so its
    matmuls/exps always beat projection filler in the ready heaps. The
    ACT exp stream starts ~23us in (vs 103us fully phased).
  - All four projections run as fp8e4 DoubleRow matmuls (weights and
    hidden states quantized host-side, x32 weight prescale to dodge the
    fp8 subnormal range; compensated via the exp scale immediate and the
    out-proj residual add). Halves projection PE streaming time.
  - Attention math stays bf16: scores as row-tiled head pairs, PV + the
    ones-matmul softmax denominator as column-tiled head pairs
    (DoubleRow is a net loss here: its dst-partition-0 ISA restriction
    breaks head packing and the psum re-plan serializes the pipeline).
  - 1/den via ACT exp(-ln(den)) (both funcs in the pinned
    natural_log_exp table set; replaces a 53us DVE RECIPROCAL).
  - LayerNorm stats via DVE bn_stats/bn_aggr; rstd via tiny ACT ops;
    fast path when ln_g==1 and ln_b==0.
  - Out-proj+LN per query tile interleaved into the second attention
    half; the last 4 tiles run a 3/4 + 1/4 two-pass contraction so only
    a sliver of work trails the final attention.

Matmul layouts (out = lhsT.T @ rhs, contraction on partitions):
  QT/KT [F, *] bf16 : lhsT=w fp8 [128, 2, 128] dc-pairs (DoubleRow),
                      rhs=hT* fp8 [128, 2, t] -> psum -> DVE cast bf16
  V     [TK, F] bf16: lhsT=hTk fp8 pairs, rhs=wv fp8 [128, 2, 512] (DR)
  S^T   [j, (h0 i512 | h1 i512)] psum: row-tiled head pair, bf16
  exp   one ACT op per j-tile: [128, 1024], bias=maskbias, scale=1/8192
  PV+den [d0|d1, i] + [den, i]: col-tiled head pairs, bf16 e and V
  O     [t, D] psum : lhsT=AVT fp8 fc-pairs (DR), rhs=wo fp8 pairs
"""
import numpy as np
import ml_dtypes

import concourse.bass as bass
import concourse.tile as tile
from concourse import bacc, mybir

F32 = mybir.dt.float32
BF16 = mybir.dt.bfloat16
FP8 = mybir.dt.float8e4
DR = mybir.MatmulPerfMode.DoubleRow
AF = mybir.ActivationFunctionType
ALU = mybir.AluOpType

NEG_BIG = -1.0e30
# exp(s + SHIFT) keeps e comfortably inside fp8e4 range (max 448); the
# uniform scale cancels in the softmax quotient PV/den.
SHIFT = -4.0 * float(np.log(2.0))
# Weights are scaled x32 on the host so fp8e4 stores them as normals
# (raw values ~+-0.03 would land in the subnormal range). Compensated by
# the exp() scale immediate (scores) and the out-proj residual add.
WS = 32.0


def _pin_act_tables():
    """Force every ACT func we use (Exp, Ln) to resolve to the single
    `natural_log_exp_and_others` table set, so the kernel does exactly
    one ACT_TABLE_LOAD instead of thrashing (~2.6us per switch)."""
    import concourse.hw_specs as hw_specs
    if getattr(hw_specs, "_mha_tables_pinned", False):
        return
    orig = hw_specs.get_activation_tables

    def patched(module_arch):
        tabs = orig(module_arch)
        pin = "natural_log_exp_and_others"
        if pin in tabs:
            pinned_funcs = tabs[pin]
            for name, fns in tabs.items():
                if name != pin:
                    tabs[name] = fns - pinned_funcs
        return tabs

    hw_specs.get_activation_tables = patched
    import concourse.bacc as bacc_mod
    bacc_mod.get_activation_tables = patched
    hw_specs._mha_tables_pinned = True


def _chunks(total, step):
    out = []
    off = 0
    while off < total:
        out.append((off, min(step, total - off)))
        off += step
    return out


def build_nc(T, TQ, TK, D, NH, DH, n_cores=8, simple_ln=True, nfp=0,
             debug=False):
    """Build the single-core SPMD Bass program. TK = compacted key count.
    simple_ln: ln_g==1 and ln_b==0, skip the gamma/beta applies.
    nfp: number of j-tile PAIRS guaranteed fully unmasked on every core
    (their exp ops fuse into one N=2048 ACT op with a constant bias)."""
    F = NH * DH
    DC = D // 128        # D contraction chunks
    FC = F // 128        # feature chunks (2 heads per chunk, DH=64)
    KC = TK // 128       # key tiles
    TT = TQ // 128       # query t-tiles
    ICS = min(512, TQ)   # i-chunk size
    ICN = TQ // ICS
    DS = min(512, D)
    FS = 512             # V-proj f-chunk width
    NFS = F // FS
    assert DH == 64 and F % 128 == 0 and D % 128 == 0
    assert TQ % 128 == 0 and TK % 128 == 0 and ICN == 2
    assert DC % 2 == 0 and FC % 2 == 0 and KC >= 2

    SC = 1.0 / (float(np.sqrt(DH)) * WS * WS)   # exp scale immediate

    _pin_act_tables()
    nc = bacc.Bacc("TRN2", target_bir_lowering=False, debug=debug,
                   num_devices=n_cores)

    # ---- DRAM I/O ----
    # wq/wk host layout: [head-pair p][128 dpart, DC*128] so each
    # head-pair's weights arrive in one contiguous DMA. wv: [fc][128,
    # DC*512]. (Host does the block transposes.)
    hTq_d = nc.dram_tensor("hTq", [DC * 128, TQ], FP8, kind="ExternalInput")
    hTk_d = nc.dram_tensor("hTk", [DC * 128, TK], FP8, kind="ExternalInput")
    hq_d = nc.dram_tensor("hq", [TQ, D], F32, kind="ExternalInput")
    wqT_d = nc.dram_tensor("wqT", [FC, 128, DC * 128], FP8,
                           kind="ExternalInput")
    wkT_d = nc.dram_tensor("wkT", [FC, 128, DC * 128], FP8,
                           kind="ExternalInput")
    wvT_d = nc.dram_tensor("wvT", [NFS, 128, DC * FS], FP8,
                           kind="ExternalInput")
    woT_d = nc.dram_tensor("woT", [FC, 128, D], FP8, kind="ExternalInput")
    mb_d = nc.dram_tensor("maskbias", [128, KC], F32, kind="ExternalInput")
    if not simple_ln:
        g_d = nc.dram_tensor("g_rep", [128, D], F32, kind="ExternalInput")
        b_d = nc.dram_tensor("b_rep", [128, D], F32, kind="ExternalInput")
    out_d = nc.dram_tensor("out", [TQ, D], F32, kind="ExternalOutput")

    with tile.TileContext(nc) as tc:
        with (
            tc.tile_pool(name="hpool", bufs=1) as hpool,
            tc.tile_pool(name="wts", bufs=1) as wts,
            tc.tile_pool(name="acts", bufs=1) as acts,
            tc.tile_pool(name="small", bufs=1) as small,
            tc.tile_pool(name="exps", bufs=10) as expp,
            tc.tile_pool(name="eln", bufs=6) as eln,
            tc.tile_pool(name="epi", bufs=3) as epi,
            tc.tile_pool(name="latex", bufs=4) as latex,
            tc.tile_pool(name="psS", bufs=2, space="PSUM") as psS,
            tc.tile_pool(name="psPV", bufs=2, space="PSUM") as psPV,
            tc.tile_pool(name="psP", bufs=2, space="PSUM") as psP,
        ):
            # ---- persistent SBUF tiles ----
            hTq = hpool.tile([128, DC * TQ], FP8, tag="htq")
            hTk = hpool.tile([128, DC * TK], FP8, tag="htk")
            # wq/wk indexed [(p*DC + dc)*128 + col]; wv [(fc*DC + dc)*FS + c]
            wqT = wts.tile([128, FC * DC * 128], FP8, tag="wq")
            wkT = wts.tile([128, FC * DC * 128], FP8, tag="wk")
            wvT = wts.tile([128, NFS * DC * FS], FP8, tag="wv")
            QT = acts.tile([128, FC * TQ], BF16, tag="qt")
            KT = acts.tile([128, FC * TK], BF16, tag="kt")
            V = acts.tile([128, KC * F], BF16, tag="v")
            AVT = acts.tile([128, FC * TQ], FP8, tag="avt")
            ones = small.tile([128, 64], BF16, tag="ones")
            mb = small.tile([128, KC], F32, tag="mb")
            eps_t = small.tile([128, 1], F32, tag="eps")
            shift_t = small.tile([128, 1], F32, tag="shift")
            if not simple_ln:
                g_re = small.tile([128, D], F32, tag="g")
                b_re = small.tile([128, D], F32, tag="b")

            nc.vector.memset(ones[:], 1.0)
            nc.vector.memset(eps_t[:], 1e-5)
            nc.vector.memset(shift_t[:], SHIFT)
            nc.sync.dma_start(mb[:], mb_d[:])

            def dma_wq(p):
                if p < FC:
                    nc.gpsimd.dma_start(
                        wqT[:, p * DC * 128:(p + 1) * DC * 128], wqT_d[p])

            def dma_wk(p):
                if p < FC:
                    nc.gpsimd.dma_start(
                        wkT[:, p * DC * 128:(p + 1) * DC * 128], wkT_d[p])

            def dma_wv(fc):
                nc.sync.dma_start(
                    wvT[:, fc * DC * FS:(fc + 1) * DC * FS], wvT_d[fc])

            # 3D views for DoubleRow operand pairs (contraction = 128
            # partitions x 2 along the named free axis).
            hTq3 = hTq[:].rearrange("p (dc t) -> p dc t", dc=DC)
            hTk3 = hTk[:].rearrange("p (dc t) -> p dc t", dc=DC)
            wq3 = wqT[:].rearrange("p (c x) -> p c x", c=FC * DC)
            wk3 = wkT[:].rearrange("p (c x) -> p c x", c=FC * DC)
            wv3 = wvT[:].rearrange("p (c x) -> p c x", c=NFS * DC)
            AVT3 = AVT[:].rearrange("p (fc t) -> p fc t", fc=FC)

            # ---- projection chain emitters (fp8 DoubleRow, dc-pairs) ----
            def q_proj(p, chunks=None):
                if p >= FC:
                    return
                for t0, tn in (chunks or _chunks(TQ, 512)):
                    ps = psP.tile([128, 512], F32, tag="P")
                    for dh in range(DC // 2):
                        nc.tensor.matmul(
                            ps[:, 0:tn],
                            wq3[:, p * DC + 2 * dh: p * DC + 2 * dh + 2, :],
                            hTq3[:, 2 * dh: 2 * dh + 2, t0:t0 + tn],
                            start=(dh == 0), stop=(dh == DC // 2 - 1),
                            perf_mode=DR)
                    nc.vector.tensor_copy(
                        QT[:, p * TQ + t0: p * TQ + t0 + tn], ps[:, 0:tn])

            def k_proj(p, chunks=None):
                if p >= FC:
                    return
                for t0, tn in (chunks or _chunks(TK, 512)):
                    ps = psP.tile([128, 512], F32, tag="P")
                    for dh in range(DC // 2):
                        nc.tensor.matmul(
                            ps[:, 0:tn],
                            wk3[:, p * DC + 2 * dh: p * DC + 2 * dh + 2, :],
                            hTk3[:, 2 * dh: 2 * dh + 2, t0:t0 + tn],
                            start=(dh == 0), stop=(dh == DC // 2 - 1),
                            perf_mode=DR)
                    nc.vector.tensor_copy(
                        KT[:, p * TK + t0: p * TK + t0 + tn], ps[:, 0:tn])

            def v_proj(jc, fc):
                f0 = fc * FS
                ps = psP.tile([128, 512], F32, tag="P")
                for dh in range(DC // 2):
                    nc.tensor.matmul(
                        ps[:],
                        hTk3[:, 2 * dh: 2 * dh + 2,
                             jc * 128:(jc + 1) * 128],
                        wv3[:, fc * DC + 2 * dh: fc * DC + 2 * dh + 2, :],
                        start=(dh == 0), stop=(dh == DC // 2 - 1),
                        perf_mode=DR)
                nc.vector.tensor_copy(
                    V[:, jc * F + f0: jc * F + f0 + FS], ps[:])

            def score_pair(s, off, hp, jc, io):
                """Emit the row-tiled score matmul pair for (hp, jc) into
                s[:, off:off+2*ICS] = [h0-i | h1-i]."""
                nc.tensor.matmul(
                    s[:, off:off + ICS],
                    KT[0:64, hp * TK + jc * 128: hp * TK + (jc + 1) * 128],
                    QT[0:64, hp * TQ + io: hp * TQ + io + ICS],
                    start=True, stop=True, tile_position=(0, 0))
                nc.tensor.matmul(
                    s[:, off + ICS:off + 2 * ICS],
                    KT[64:128, hp * TK + jc * 128: hp * TK + (jc + 1) * 128],
                    QT[64:128, hp * TQ + io: hp * TQ + io + ICS],
                    start=True, stop=True, tile_position=(64, 0))

            def attention(ic, hp):
                with tc.high_priority(offset=500000):
                    _attention(ic, hp)

            def _attention(ic, hp):
                io = ic * ICS
                h0, h1 = 2 * hp, 2 * hp + 1
                pvP = psPV.tile([128, ICS], F32, tag="pv")
                pvD = psPV.tile([128, ICS], F32, tag="pv")
                for jc in range(KC):
                    s = psS.tile([128, 2 * ICS], F32, tag="S")
                    score_pair(s, 0, hp, jc, io)
                    e = expp.tile([128, 2 * ICS], BF16, tag="e")
                    nc.scalar.activation(e[:], s[:], AF.Exp,
                                         bias=mb[:, jc:jc + 1], scale=SC)
                    st, sp = (jc == 0), (jc == KC - 1)
                    nc.tensor.matmul(
                        pvP[0:64, :],
                        V[:, jc * F + h0 * DH: jc * F + (h0 + 1) * DH],
                        e[:, 0:ICS], start=st, stop=sp,
                        tile_position=(0, 0), skip_group_check=True)
                    nc.tensor.matmul(
                        pvP[64:128, :],
                        V[:, jc * F + h1 * DH: jc * F + (h1 + 1) * DH],
                        e[:, ICS:2 * ICS], start=st, stop=sp,
                        tile_position=(0, 64), skip_group_check=True)
                    nc.tensor.matmul(
                        pvD[0:64, :], ones[:, 0:64],
                        e[:, 0:ICS], start=st, stop=sp,
                        tile_position=(0, 0), skip_group_check=True)
                    nc.tensor.matmul(
                        pvD[64:128, :], ones[:, 0:64],
                        e[:, ICS:2 * ICS], start=st, stop=sp,
                        tile_position=(0, 64), skip_group_check=True)
                return_normalize(hp, io, pvP, pvD)

            def return_normalize(hp, io, pvP, pvD):
                # 1/den = exp(-ln(den)) on ACT (stays in the pinned table
                # set); then one DVE multiply straight out of PSUM.
                lden = eln.tile([128, ICS], F32, tag="r")
                nc.scalar.activation(lden[:], pvD[:], AF.Ln)
                rden = eln.tile([128, ICS], F32, tag="r")
                nc.scalar.activation(rden[:], lden[:], AF.Exp, scale=-1.0)
                nc.vector.tensor_tensor(
                    AVT[:, hp * TQ + io: hp * TQ + io + ICS],
                    pvP[:], rden[:], op=ALU.mult)

            def oln_head(tt, fh0, fh1, x=None, hqt=None, pool=None):
                """O-proj partial over fc pairs [fh0, fh1) + residual/accum
                into x. Returns (x, hqt)."""
                first = x is None
                if first:
                    pool = pool or epi
                    x = pool.tile([128, D], F32, tag="x")
                    hqt = pool.tile([128, D], F32, tag="hqt")
                    nc.sync.dma_start(hqt[:],
                                      hq_d[tt * 128:(tt + 1) * 128, :])
                for d0, dn in _chunks(D, DS):
                    ps = psP.tile([128, 512], F32, tag="P")
                    n = len(fh0) if isinstance(fh0, list) else 0
                    parts = fh0 if n else [("pair", fh) for fh in
                                           range(fh0, fh1)]
                    for i, (kind, fc_) in enumerate(parts):
                        st, sp = (i == 0), (i == len(parts) - 1)
                        if kind == "pair":
                            nc.tensor.matmul(
                                ps[:, 0:dn],
                                AVT3[:, 2 * fc_:2 * fc_ + 2,
                                     tt * 128:(tt + 1) * 128],
                                wo3[:, 2 * fc_:2 * fc_ + 2, d0:d0 + dn],
                                start=st, stop=sp, perf_mode=DR)
                        else:
                            nc.tensor.matmul(
                                ps[:, 0:dn],
                                AVT[:, fc_ * TQ + tt * 128:
                                    fc_ * TQ + (tt + 1) * 128],
                                woT[:, fc_ * D + d0: fc_ * D + d0 + dn],
                                start=st, stop=sp)
                    nc.vector.scalar_tensor_tensor(
                        x[:, d0:d0 + dn], ps[:, 0:dn], 1.0 / (WS * WS),
                        (hqt if first else x)[:, d0:d0 + dn],
                        op0=ALU.mult, op1=ALU.add)
                return x, hqt

            def oln_ln(tt, x):
                # mean/var in one DVE pass (bn hardware limit: 512/group)
                stats = epi.tile([128, 12], F32, tag="st")
                nc.vector.bn_stats(stats[:, 0:6], x[:, 0:512])
                nc.vector.bn_stats(stats[:, 6:12], x[:, 512:1024])
                mv = epi.tile([128, 4], F32, tag="mv")
                nc.vector.bn_aggr(mv[:, 0:2], stats[:])
                # rstd = exp(-0.5*ln(var+eps)); Ln+Exp stay in-table-set
                lnv = mv[:, 2:3]
                nc.scalar.activation(lnv, mv[:, 1:2], AF.Ln, bias=eps_t[:])
                rstd = mv[:, 3:4]
                nc.scalar.activation(rstd, lnv, AF.Exp, scale=-0.5)
                # s2 = -mu*rstd;  x = x*rstd + s2  (per-partition affine)
                s2 = epi.tile([128, 1], F32, tag="s2")
                nc.vector.scalar_tensor_tensor(
                    s2[:], mv[:, 0:1], -1.0, rstd, op0=ALU.mult, op1=ALU.mult)
                nc.vector.tensor_scalar(x[:], x[:], rstd, s2[:],
                                        op0=ALU.mult, op1=ALU.add)
                if not simple_ln:
                    nc.vector.tensor_tensor(x[:], x[:], g_re[:], op=ALU.mult)
                    nc.vector.tensor_tensor(x[:], x[:], b_re[:], op=ALU.add)
                nc.sync.dma_start(out_d[tt * 128:(tt + 1) * 128, :], x[:])

            def oln_tile(tt):
                x, hqt = oln_head(tt, 0, FC // 4)
                x, _ = oln_head(tt, FC // 4, FC // 2, x, hqt)
                oln_ln(tt, x)

            # ---- DMA head-start + stage-1 projections for the pipeline ----
            dma_wk(0)
            for dc in range(DC):
                eng = nc.sync if dc % 2 == 0 else nc.gpsimd
                eng.dma_start(hTk[:, dc * TK:(dc + 1) * TK],
                              hTk_d[dc * 128:(dc + 1) * 128, :])
            dma_wq(0)
            for dc in range(DC):
                eng = nc.gpsimd if dc % 2 == 0 else nc.sync
                eng.dma_start(hTq[:, dc * TQ:(dc + 1) * TQ],
                              hTq_d[dc * 128:(dc + 1) * 128, :])
            dma_wk(1)
            dma_wq(1)
            dma_wv(0)
            if not simple_ln:
                nc.sync.dma_start(g_re[:], g_d[:])
                nc.sync.dma_start(b_re[:], b_d[:])

            # first chunks only: unblocks scores (hp0, ic0, jc0-3) ASAP
            k_proj(0, chunks=[(0, 512)])
            q_proj(0, chunks=[(0, 512)])
            k_proj(0, chunks=_chunks(TK, 512)[1:])
            q_proj(0, chunks=_chunks(TQ, 512)[1:])
            for jc in range(KC):
                v_proj(jc, 0)
            dma_wq(2)
            dma_wk(2)
            q_proj(1)
            k_proj(1)

            # ---- pipelined main loop ----
            # ic0: attention per head-pair; the next head-pairs' proj (and
            # V f-chunk 1, late DMAs) emitted behind it as PE filler.
            woT = wo3 = None
            for hp in range(FC):
                attention(0, hp)
                dma_wq(hp + 3)
                dma_wk(hp + 3)
                q_proj(hp + 2)
                k_proj(hp + 2)
                if hp == 1:
                    dma_wv(1)
                    for jc in range(KC):
                        v_proj(jc, 1)
                if hp == 5:
                    # woT reuses wq's SBUF slot (wq is dead after
                    # q_proj(7), emitted this iteration).
                    woT = wts.tile([128, FC * D], FP8, tag="wq")
                    wo3 = woT[:].rearrange("p (fc d) -> p fc d", fc=FC)
                    for fc_ in range(FC):
                        nc.sync.dma_start(woT[:, fc_ * D:(fc_ + 1) * D],
                                          woT_d[fc_])
            # ic1: attention with ic0's out-proj+LN tiles as filler.
            late = {}
            for hp in range(FC):
                attention(1, hp)
                if hp % 2 == 1:
                    oln_tile(hp // 2)
                if hp == FC - 3:
                    # O-proj of the tail tiles over all but the last
                    # fc-pair becomes filler; only 1/4 remains at the end
                    for tt in range(TT // 2, TT):
                        late[tt] = oln_head(tt, 0, FC // 2 - 1, pool=latex)
                if hp == FC - 2:
                    for tt in range(TT // 2, TT):
                        late[tt] = oln_head(
                            tt, [("single", FC - 2)], None, *late[tt])
            for tt in range(TT // 2, TT):
                x, _ = oln_head(tt, [("single", FC - 1)], None, *late[tt])
                oln_ln(tt, x)

    nc.compile()
    return nc


def choose_tk(attn_mask):
    """Compacted key count: max unmasked count over batches, ceil to 128."""
    m = np.asarray(attn_mask)
    counts = (~m).sum(axis=0)
    tk = int(((int(counts.max()) + 127) // 128) * 128)
    return max(tk, 128)


def host_prep_core(c, tk, h, attn_mask, wq, wkv, wo, ln_g, ln_b,
                   NH=16, DH=64, simple_ln=True):
    """Build the per-core input map (numpy) for core c."""
    T, B, D = h.shape
    F = NH * DH
    TQ = T // 2
    KC = tk // 128
    DC = D // 128
    FC = F // 128
    NFS = F // 512
    b, qh = c // 2, c % 2
    f8 = ml_dtypes.float8_e4m3
    hb = np.roll(np.asarray(h[:, b, :], dtype=np.float32), -qh * TQ, axis=0)
    maskb = np.roll(np.asarray(attn_mask[:, b]), -qh * TQ)
    idx = np.nonzero(~maskb)[0]
    nk = idx.shape[0]
    assert nk <= tk
    idxp = np.concatenate([idx, np.zeros(tk - nk, np.int64)])
    hbT = np.ascontiguousarray(hb.T).astype(f8)             # [D, T]
    m = {}
    m["hTq"] = np.ascontiguousarray(hbT[:, :TQ])
    m["hTk"] = np.ascontiguousarray(hbT[:, idxp])
    m["hq"] = np.ascontiguousarray(hb[:TQ])                 # [TQ, D] f32
    # [D, F] -> [FC][128, DC*128]: block (p, dc) at [:, dc*128:...] holds
    # wT[dc*128:(dc+1)*128, p*128:(p+1)*128]
    wqT = (wq.T * WS).astype(f8)                            # [D, F]
    wkT = (wkv[:F].T * WS).astype(f8)
    wvT = (wkv[F:].T * WS).astype(f8)
    m["wqT"] = np.ascontiguousarray(
        wqT.reshape(DC, 128, FC, 128).transpose(2, 1, 0, 3)
           .reshape(FC, 128, DC * 128))
    m["wkT"] = np.ascontiguousarray(
        wkT.reshape(DC, 128, FC, 128).transpose(2, 1, 0, 3)
           .reshape(FC, 128, DC * 128))
    m["wvT"] = np.ascontiguousarray(
        wvT.reshape(DC, 128, NFS, 512).transpose(2, 1, 0, 3)
           .reshape(NFS, 128, DC * 512))
    m["woT"] = np.ascontiguousarray((wo.T * WS).astype(f8).reshape(FC, 128, D))
    mbias = np.full(tk, NEG_BIG, np.float32)
    mbias[:nk] = SHIFT
    m["maskbias"] = np.ascontiguousarray(mbias.reshape(KC, 128).T)
    if not simple_ln:
        m["g_rep"] = np.ascontiguousarray(
            np.broadcast_to(np.asarray(ln_g, np.float32), (128, D)))
        m["b_rep"] = np.ascontiguousarray(
            np.broadcast_to(np.asarray(ln_b, np.float32), (128, D)))
    return m

# ======================================================================
# Host-side runner: shard, compile (cached), execute on 8 cores, gather.
# ======================================================================
_NC_CACHE = {}
LAST_RESULT = None  # BassKernelResults of the most recent kernel() call


def _get_nc(T, TQ, TK, D, NH, DH, simple_ln, nfp):
    key = (T, TQ, TK, D, NH, DH, simple_ln, nfp)
    if key not in _NC_CACHE:
        _NC_CACHE[key] = build_nc(T, TQ, TK, D, NH, DH, n_cores=8,
                                  simple_ln=simple_ln, nfp=nfp, debug=False)
    return _NC_CACHE[key]


def kernel(h, attn_mask, wq, wkv, wo, ln_g, ln_b):
    """Full-input MultiHeadAttn forward on 8 NeuronCores.

    h: [T, B, D] f32; attn_mask: [T, B] bool (True = masked key);
    wq: [F, D]; wkv: [2F, D]; wo: [D, F]; ln_g/ln_b: [D].
    Returns [T, B, D] f32 = layer_norm(h + attn(h)).
    """
    from concourse.bass_utils import run_bass_kernel_spmd
    global LAST_RESULT

    h = np.asarray(h)
    attn_mask = np.asarray(attn_mask)
    wq = np.asarray(wq, np.float32)
    wkv = np.asarray(wkv, np.float32)
    wo = np.asarray(wo, np.float32)
    ln_g = np.asarray(ln_g, np.float32)
    ln_b = np.asarray(ln_b, np.float32)

    T, B, D = h.shape
    NH = 16
    DH = wq.shape[0] // NH
    assert 2 * B == 8, "sharding assumes batch 4 over 8 cores"
    TQ = T // 2
    TK = min(choose_tk(attn_mask), T)
    simple_ln = bool(np.all(ln_g == 1.0) and np.all(ln_b == 0.0))
    # j-tile pairs fully unmasked on every core (exp ops fuse over them):
    # per-batch unmasked counts are identical for both query-half cores.
    nk_min = int((~attn_mask).sum(axis=0).min())
    n_full = nk_min // 128          # leading fully-unmasked j-tiles
    nfp = max(0, min(TK // 256, n_full // 2))

    nc = _get_nc(T, TQ, TK, D, NH, DH, simple_ln, nfp)
    in_maps = [host_prep_core(c, TK, h, attn_mask, wq, wkv, wo, ln_g, ln_b,
                              NH=NH, DH=DH, simple_ln=simple_ln)
               for c in range(8)]
    res = run_bass_kernel_spmd(nc, in_maps, core_ids=list(range(8)))
    LAST_RESULT = res

    out = np.empty((T, B, D), np.float32)
    for c in range(8):
        b, qh = c // 2, c % 2
        out[qh * TQ:(qh + 1) * TQ, b, :] = res.results[c]["out"]
    return out


# revision 31
# speedup vs baseline: 1.0059x; 1.0059x over previous
"""Multi-head attention Bass/Tile kernel for TRN2, 8-core SPMD.

Sharding: core c handles batch b = c//2, query-half qh = c%2. The host
rotates the token axis per core so query rows sit at [0:TQ] (attention is
key-permutation invariant), and gathers the unmasked keys (mask compaction)
so K/V projection + attention only touch TK <= T key tokens.

Architecture (measured ~257us vs 378us baseline):
  - Software-pipelined emission: Tile's scheduler is an out-of-order
    per-engine list scheduler (priority = emission order), so Q/K/V
    projections are emitted per head-pair interleaved with the attention
    loop; attention itself is emitted under tc.high_priority so its
    matmuls/exps always beat projection filler in the ready heaps. The
    ACT exp stream starts ~23us in (vs 103us fully phased).
  - All four projections run as fp8e4 DoubleRow matmuls (weights and
    hidden states quantized host-side, x32 weight prescale to dodge the
    fp8 subnormal range; compensated via the exp scale immediate and the
    out-proj residual add). Halves projection PE streaming time.
  - Attention math stays bf16: scores as row-tiled head pairs, PV + the
    ones-matmul softmax denominator as column-tiled head pairs
    (DoubleRow is a net loss here: its dst-partition-0 ISA restriction
    breaks head packing and the psum re-plan serializes the pipeline).
  - 1/den via ACT exp(-ln(den)) (both funcs in the pinned
    natural_log_exp table set; replaces a 53us DVE RECIPROCAL).
  - LayerNorm stats via DVE bn_stats/bn_aggr; rstd via tiny ACT ops;
    fast path when ln_g==1 and ln_b==0.
  - Out-proj+LN per query tile interleaved into the second attention
    half; the last 4 tiles run a 3/4 + 1/4 two-pass contraction so only
    a sliver of work trails the final attention.

Matmul layouts (out = lhsT.T @ rhs, contraction on partitions):
  QT/KT [F, *] bf16 : lhsT=w fp8 [128, 2, 128] dc-pairs (DoubleRow),
                      rhs=hT* fp8 [128, 2, t] -> psum -> DVE cast bf16
  V     [TK, F] bf16: lhsT=hTk fp8 pairs, rhs=wv fp8 [128, 2, 512] (DR)
  S^T   [j, (h0 i512 | h1 i512)] psum: row-tiled head pair, bf16
  exp   one ACT op per j-tile: [128, 1024], bias=maskbias, scale=1/8192
  PV+den [d0|d1, i] + [den, i]: col-tiled head pairs, bf16 e and V
  O     [t, D] psum : lhsT=AVT fp8 fc-pairs (DR), rhs=wo fp8 pairs
"""
import numpy as np
import ml_dtypes

import concourse.bass as bass
import concourse.tile as tile
from concourse import bacc, mybir

F32 = mybir.dt.float32
BF16 = mybir.dt.bfloat16
FP8 = mybir.dt.float8e4
DR = mybir.MatmulPerfMode.DoubleRow
AF = mybir.ActivationFunctionType
ALU = mybir.AluOpType

NEG_BIG = -1.0e30
# exp(s + SHIFT) keeps e comfortably inside fp8e4 range (max 448); the
# uniform scale cancels in the softmax quotient PV/den.
SHIFT = -4.0 * float(np.log(2.0))
# Weights are scaled x32 on the host so fp8e4 stores them as normals
# (raw values ~+-0.03 would land in the subnormal range). Compensated by
# the exp() scale immediate (scores) and the out-proj residual add.
WS = 32.0


def _pin_act_tables():
    """Force every ACT func we use (Exp, Ln) to resolve to the single
    `natural_log_exp_and_others` table set, so the kernel does exactly
    one ACT_TABLE_LOAD instead of thrashing (~2.6us per switch)."""
    import concourse.hw_specs as hw_specs
    if getattr(hw_specs, "_mha_tables_pinned", False):
        return
    orig = hw_specs.get_activation_tables

    def patched(module_arch):
        tabs = orig(module_arch)
        pin = "natural_log_exp_and_others"
        if pin in tabs:
            pinned_funcs = tabs[pin]
            for name, fns in tabs.items():
                if name != pin:
                    tabs[name] = fns - pinned_funcs
        return tabs

    hw_specs.get_activation_tables = patched
    import concourse.bacc as bacc_mod
    bacc_mod.get_activation_tables = patched
    hw_specs._mha_tables_pinned = True


def _chunks(total, step):
    out = []
    off = 0
    while off < total:
        out.append((off, min(step, total - off)))
        off += step
    return out


def build_nc(T, TQ, TK, D, NH, DH, n_cores=8, simple_ln=True, nfp=0,
             debug=False):
    """Build the single-core SPMD Bass program. TK = compacted key count.
    simple_ln: ln_g==1 and ln_b==0, skip the gamma/beta applies.
    nfp: number of j-tile PAIRS guaranteed fully unmasked on every core
    (their exp ops fuse into one N=2048 ACT op with a constant bias)."""
    F = NH * DH
    DC = D // 128        # D contraction chunks
    FC = F // 128        # feature chunks (2 heads per chunk, DH=64)
    KC = TK // 128       # key tiles
    TT = TQ // 128       # query t-tiles
    ICS = min(512, TQ)   # i-chunk size
    ICN = TQ // ICS
    DS = min(512, D)
    FS = 512             # V-proj f-chunk width
    NFS = F // FS
    assert DH == 64 and F % 128 == 0 and D % 128 == 0
    assert TQ % 128 == 0 and TK % 128 == 0 and ICN == 2
    assert DC % 2 == 0 and FC % 2 == 0 and KC >= 2

    SC = 1.0 / (float(np.sqrt(DH)) * WS * WS)   # exp scale immediate

    _pin_act_tables()
    nc = bacc.Bacc("TRN2", target_bir_lowering=False, debug=debug,
                   num_devices=n_cores)

    # ---- DRAM I/O ----
    # wq/wk host layout: [head-pair p][128 dpart, DC*128] so each
    # head-pair's weights arrive in one contiguous DMA. wv: [fc][128,
    # DC*512]. (Host does the block transposes.)
    hTq_d = nc.dram_tensor("hTq", [128, DC * TQ], FP8, kind="ExternalInput")
    hTk_d = nc.dram_tensor("hTk", [128, DC * TK], FP8, kind="ExternalInput")
    hq_d = nc.dram_tensor("hq", [TQ, D], F32, kind="ExternalInput")
    wqT_d = nc.dram_tensor("wqT", [FC, 128, DC * 128], FP8,
                           kind="ExternalInput")
    wkT_d = nc.dram_tensor("wkT", [FC, 128, DC * 128], FP8,
                           kind="ExternalInput")
    wvT_d = nc.dram_tensor("wvT", [NFS, 128, DC * FS], FP8,
                           kind="ExternalInput")
    woT_d = nc.dram_tensor("woT", [FC, 128, D], FP8, kind="ExternalInput")
    mb_d = nc.dram_tensor("maskbias", [128, KC], F32, kind="ExternalInput")
    if not simple_ln:
        g_d = nc.dram_tensor("g_rep", [128, D], F32, kind="ExternalInput")
        b_d = nc.dram_tensor("b_rep", [128, D], F32, kind="ExternalInput")
    out_d = nc.dram_tensor("out", [TQ, D], F32, kind="ExternalOutput")

    with tile.TileContext(nc) as tc:
        with (
            tc.tile_pool(name="hpool", bufs=1) as hpool,
            tc.tile_pool(name="wts", bufs=1) as wts,
            tc.tile_pool(name="acts", bufs=1) as acts,
            tc.tile_pool(name="small", bufs=1) as small,
            tc.tile_pool(name="exps", bufs=10) as expp,
            tc.tile_pool(name="eln", bufs=6) as eln,
            tc.tile_pool(name="epi", bufs=3) as epi,
            tc.tile_pool(name="latex", bufs=4) as latex,
            tc.tile_pool(name="psS", bufs=2, space="PSUM") as psS,
            tc.tile_pool(name="psPV", bufs=2, space="PSUM") as psPV,
            tc.tile_pool(name="psP", bufs=2, space="PSUM") as psP,
        ):
            # ---- persistent SBUF tiles ----
            hTq = hpool.tile([128, DC * TQ], FP8, tag="htq")
            hTk = hpool.tile([128, DC * TK], FP8, tag="htk")
            # wq/wk indexed [(p*DC + dc)*128 + col]; wv [(fc*DC + dc)*FS + c]
            wqT = wts.tile([128, FC * DC * 128], FP8, tag="wq")
            wkT = wts.tile([128, FC * DC * 128], FP8, tag="wk")
            wvT = wts.tile([128, NFS * DC * FS], FP8, tag="wv")
            QT = acts.tile([128, FC * TQ], BF16, tag="qt")
            KT = acts.tile([128, FC * TK], BF16, tag="kt")
            V = acts.tile([128, KC * F], BF16, tag="v")
            AVT = acts.tile([128, FC * TQ], FP8, tag="avt")
            ones = small.tile([128, 64], BF16, tag="ones")
            mb = small.tile([128, KC], F32, tag="mb")
            eps_t = small.tile([128, 1], F32, tag="eps")
            shift_t = small.tile([128, 1], F32, tag="shift")
            if not simple_ln:
                g_re = small.tile([128, D], F32, tag="g")
                b_re = small.tile([128, D], F32, tag="b")

            nc.vector.memset(ones[:], 1.0)
            nc.vector.memset(eps_t[:], 1e-5)
            nc.vector.memset(shift_t[:], SHIFT)
            nc.sync.dma_start(mb[:], mb_d[:])

            def dma_wq(p):
                if p < FC:
                    nc.gpsimd.dma_start(
                        wqT[:, p * DC * 128:(p + 1) * DC * 128], wqT_d[p])

            def dma_wk(p):
                if p < FC:
                    nc.gpsimd.dma_start(
                        wkT[:, p * DC * 128:(p + 1) * DC * 128], wkT_d[p])

            def dma_wv(fc):
                nc.sync.dma_start(
                    wvT[:, fc * DC * FS:(fc + 1) * DC * FS], wvT_d[fc])

            # 3D views for DoubleRow operand pairs (contraction = 128
            # partitions x 2 along the named free axis).
            hTq3 = hTq[:].rearrange("p (dc t) -> p dc t", dc=DC)
            hTk3 = hTk[:].rearrange("p (dc t) -> p dc t", dc=DC)
            wq3 = wqT[:].rearrange("p (c x) -> p c x", c=FC * DC)
            wk3 = wkT[:].rearrange("p (c x) -> p c x", c=FC * DC)
            wv3 = wvT[:].rearrange("p (c x) -> p c x", c=NFS * DC)
            AVT3 = AVT[:].rearrange("p (fc t) -> p fc t", fc=FC)

            # ---- projection chain emitters (fp8 DoubleRow, dc-pairs) ----
            def q_proj(p, chunks=None):
                if p >= FC:
                    return
                for t0, tn in (chunks or _chunks(TQ, 512)):
                    ps = psP.tile([128, 512], F32, tag="P")
                    for dh in range(DC // 2):
                        nc.tensor.matmul(
                            ps[:, 0:tn],
                            wq3[:, p * DC + 2 * dh: p * DC + 2 * dh + 2, :],
                            hTq3[:, 2 * dh: 2 * dh + 2, t0:t0 + tn],
                            start=(dh == 0), stop=(dh == DC // 2 - 1),
                            perf_mode=DR)
                    nc.vector.tensor_copy(
                        QT[:, p * TQ + t0: p * TQ + t0 + tn], ps[:, 0:tn])

            def k_proj(p, chunks=None):
                if p >= FC:
                    return
                for t0, tn in (chunks or _chunks(TK, 512)):
                    ps = psP.tile([128, 512], F32, tag="P")
                    for dh in range(DC // 2):
                        nc.tensor.matmul(
                            ps[:, 0:tn],
                            wk3[:, p * DC + 2 * dh: p * DC + 2 * dh + 2, :],
                            hTk3[:, 2 * dh: 2 * dh + 2, t0:t0 + tn],
                            start=(dh == 0), stop=(dh == DC // 2 - 1),
                            perf_mode=DR)
                    nc.vector.tensor_copy(
                        KT[:, p * TK + t0: p * TK + t0 + tn], ps[:, 0:tn])

            def v_proj(jc, fc):
                f0 = fc * FS
                ps = psP.tile([128, 512], F32, tag="P")
                for dh in range(DC // 2):
                    nc.tensor.matmul(
                        ps[:],
                        hTk3[:, 2 * dh: 2 * dh + 2,
                             jc * 128:(jc + 1) * 128],
                        wv3[:, fc * DC + 2 * dh: fc * DC + 2 * dh + 2, :],
                        start=(dh == 0), stop=(dh == DC // 2 - 1),
                        perf_mode=DR)
                nc.vector.tensor_copy(
                    V[:, jc * F + f0: jc * F + f0 + FS], ps[:])

            def score_pair(s, off, hp, jc, io):
                """Emit the row-tiled score matmul pair for (hp, jc) into
                s[:, off:off+2*ICS] = [h0-i | h1-i]."""
                nc.tensor.matmul(
                    s[:, off:off + ICS],
                    KT[0:64, hp * TK + jc * 128: hp * TK + (jc + 1) * 128],
                    QT[0:64, hp * TQ + io: hp * TQ + io + ICS],
                    start=True, stop=True, tile_position=(0, 0))
                nc.tensor.matmul(
                    s[:, off + ICS:off + 2 * ICS],
                    KT[64:128, hp * TK + jc * 128: hp * TK + (jc + 1) * 128],
                    QT[64:128, hp * TQ + io: hp * TQ + io + ICS],
                    start=True, stop=True, tile_position=(64, 0))

            def attention(ic, hp):
                with tc.high_priority(offset=500000):
                    _attention(ic, hp)

            def _attention(ic, hp):
                io = ic * ICS
                h0, h1 = 2 * hp, 2 * hp + 1
                pvP = psPV.tile([128, ICS], F32, tag="pv")
                pvD = psPV.tile([128, ICS], F32, tag="pv")
                for jc in range(KC):
                    s = psS.tile([128, 2 * ICS], F32, tag="S")
                    score_pair(s, 0, hp, jc, io)
                    e = expp.tile([128, 2 * ICS], BF16, tag="e")
                    nc.scalar.activation(e[:], s[:], AF.Exp,
                                         bias=mb[:, jc:jc + 1], scale=SC)
                    st, sp = (jc == 0), (jc == KC - 1)
                    nc.tensor.matmul(
                        pvP[0:64, :],
                        V[:, jc * F + h0 * DH: jc * F + (h0 + 1) * DH],
                        e[:, 0:ICS], start=st, stop=sp,
                        tile_position=(0, 0), skip_group_check=True)
                    nc.tensor.matmul(
                        pvP[64:128, :],
                        V[:, jc * F + h1 * DH: jc * F + (h1 + 1) * DH],
                        e[:, ICS:2 * ICS], start=st, stop=sp,
                        tile_position=(0, 64), skip_group_check=True)
                    nc.tensor.matmul(
                        pvD[0:64, :], ones[:, 0:64],
                        e[:, 0:ICS], start=st, stop=sp,
                        tile_position=(0, 0), skip_group_check=True)
                    nc.tensor.matmul(
                        pvD[64:128, :], ones[:, 0:64],
                        e[:, ICS:2 * ICS], start=st, stop=sp,
                        tile_position=(0, 64), skip_group_check=True)
                return_normalize(hp, io, pvP, pvD)

            def return_normalize(hp, io, pvP, pvD):
                # 1/den = exp(-ln(den)) on ACT (stays in the pinned table
                # set); then one DVE multiply straight out of PSUM.
                lden = eln.tile([128, ICS], F32, tag="r")
                nc.scalar.activation(lden[:], pvD[:], AF.Ln)
                rden = eln.tile([128, ICS], F32, tag="r")
                nc.scalar.activation(rden[:], lden[:], AF.Exp, scale=-1.0)
                nc.vector.tensor_tensor(
                    AVT[:, hp * TQ + io: hp * TQ + io + ICS],
                    pvP[:], rden[:], op=ALU.mult)

            def oln_head(tt, fh0, fh1, x=None, hqt=None, pool=None):
                """O-proj partial over fc pairs [fh0, fh1) + residual/accum
                into x. Returns (x, hqt)."""
                first = x is None
                if first:
                    pool = pool or epi
                    x = pool.tile([128, D], F32, tag="x")
                    hqt = pool.tile([128, D], F32, tag="hqt")
                    nc.sync.dma_start(hqt[:],
                                      hq_d[tt * 128:(tt + 1) * 128, :])
                for d0, dn in _chunks(D, DS):
                    ps = psP.tile([128, 512], F32, tag="P")
                    n = len(fh0) if isinstance(fh0, list) else 0
                    parts = fh0 if n else [("pair", fh) for fh in
                                           range(fh0, fh1)]
                    for i, (kind, fc_) in enumerate(parts):
                        st, sp = (i == 0), (i == len(parts) - 1)
                        if kind == "pair":
                            nc.tensor.matmul(
                                ps[:, 0:dn],
                                AVT3[:, 2 * fc_:2 * fc_ + 2,
                                     tt * 128:(tt + 1) * 128],
                                wo3[:, 2 * fc_:2 * fc_ + 2, d0:d0 + dn],
                                start=st, stop=sp, perf_mode=DR)
                        else:
                            nc.tensor.matmul(
                                ps[:, 0:dn],
                                AVT[:, fc_ * TQ + tt * 128:
                                    fc_ * TQ + (tt + 1) * 128],
                                woT[:, fc_ * D + d0: fc_ * D + d0 + dn],
                                start=st, stop=sp)
                    nc.vector.scalar_tensor_tensor(
                        x[:, d0:d0 + dn], ps[:, 0:dn], 1.0 / (WS * WS),
                        (hqt if first else x)[:, d0:d0 + dn],
                        op0=ALU.mult, op1=ALU.add)
                return x, hqt

            def oln_ln(tt, x):
                # mean/var in one DVE pass (bn hardware limit: 512/group)
                stats = epi.tile([128, 12], F32, tag="st")
                nc.vector.bn_stats(stats[:, 0:6], x[:, 0:512])
                nc.vector.bn_stats(stats[:, 6:12], x[:, 512:1024])
                mv = epi.tile([128, 4], F32, tag="mv")
                nc.vector.bn_aggr(mv[:, 0:2], stats[:])
                # rstd = exp(-0.5*ln(var+eps)); Ln+Exp stay in-table-set
                lnv = mv[:, 2:3]
                nc.scalar.activation(lnv, mv[:, 1:2], AF.Ln, bias=eps_t[:])
                rstd = mv[:, 3:4]
                nc.scalar.activation(rstd, lnv, AF.Exp, scale=-0.5)
                # s2 = -mu*rstd;  x = x*rstd + s2  (per-partition affine)
                s2 = epi.tile([128, 1], F32, tag="s2")
                nc.vector.scalar_tensor_tensor(
                    s2[:], mv[:, 0:1], -1.0, rstd, op0=ALU.mult, op1=ALU.mult)
                nc.vector.tensor_scalar(x[:], x[:], rstd, s2[:],
                                        op0=ALU.mult, op1=ALU.add)
                if not simple_ln:
                    nc.vector.tensor_tensor(x[:], x[:], g_re[:], op=ALU.mult)
                    nc.vector.tensor_tensor(x[:], x[:], b_re[:], op=ALU.add)
                nc.sync.dma_start(out_d[tt * 128:(tt + 1) * 128, :], x[:])

            def oln_tile(tt):
                x, hqt = oln_head(tt, 0, FC // 4)
                x, _ = oln_head(tt, FC // 4, FC // 2, x, hqt)
                oln_ln(tt, x)

            # ---- DMA head-start + stage-1 projections for the pipeline ----
            dma_wk(0)
            nc.sync.dma_start(hTk[:], hTk_d[:])
            dma_wq(0)
            nc.gpsimd.dma_start(hTq[:], hTq_d[:])
            dma_wk(1)
            dma_wq(1)
            dma_wv(0)
            if not simple_ln:
                nc.sync.dma_start(g_re[:], g_d[:])
                nc.sync.dma_start(b_re[:], b_d[:])

            # first chunks only: unblocks scores (hp0, ic0, jc0-3) ASAP
            k_proj(0, chunks=[(0, 512)])
            q_proj(0, chunks=[(0, 512)])
            k_proj(0, chunks=_chunks(TK, 512)[1:])
            q_proj(0, chunks=_chunks(TQ, 512)[1:])
            for jc in range(KC):
                v_proj(jc, 0)
            dma_wq(2)
            dma_wk(2)
            q_proj(1)
            k_proj(1)

            # ---- pipelined main loop ----
            # ic0: attention per head-pair; the next head-pairs' proj (and
            # V f-chunk 1, late DMAs) emitted behind it as PE filler.
            woT = wo3 = None
            for hp in range(FC):
                attention(0, hp)
                dma_wq(hp + 3)
                dma_wk(hp + 3)
                q_proj(hp + 2)
                k_proj(hp + 2)
                if hp == 1:
                    dma_wv(1)
                    for jc in range(KC):
                        v_proj(jc, 1)
                if hp == 5:
                    # woT reuses wq's SBUF slot (wq is dead after
                    # q_proj(7), emitted this iteration).
                    woT = wts.tile([128, FC * D], FP8, tag="wq")
                    wo3 = woT[:].rearrange("p (fc d) -> p fc d", fc=FC)
                    for fc_ in range(FC):
                        nc.sync.dma_start(woT[:, fc_ * D:(fc_ + 1) * D],
                                          woT_d[fc_])
            # ic1: attention with ic0's out-proj+LN tiles as filler.
            late = {}
            for hp in range(FC):
                attention(1, hp)
                if hp % 2 == 1:
                    oln_tile(hp // 2)
                if hp == FC - 3:
                    # O-proj of the tail tiles over all but the last
                    # fc-pair becomes filler; only 1/4 remains at the end
                    for tt in range(TT // 2, TT):
                        late[tt] = oln_head(tt, 0, FC // 2 - 1, pool=latex)

            for tt in range(TT // 2, TT):
                x, _ = oln_head(tt, FC // 2 - 1, FC // 2, *late[tt])
                oln_ln(tt, x)

    nc.compile()
    return nc


def choose_tk(attn_mask):
    """Compacted key count: max unmasked count over batches, ceil to 128."""
    m = np.asarray(attn_mask)
    counts = (~m).sum(axis=0)
    tk = int(((int(counts.max()) + 127) // 128) * 128)
    return max(tk, 128)


def host_prep_core(c, tk, h, attn_mask, wq, wkv, wo, ln_g, ln_b,
                   NH=16, DH=64, simple_ln=True):
    """Build the per-core input map (numpy) for core c."""
    T, B, D = h.shape
    F = NH * DH
    TQ = T // 2
    KC = tk // 128
    DC = D // 128
    FC = F // 128
    NFS = F // 512
    b, qh = c // 2, c % 2
    f8 = ml_dtypes.float8_e4m3
    hb = np.roll(np.asarray(h[:, b, :], dtype=np.float32), -qh * TQ, axis=0)
    maskb = np.roll(np.asarray(attn_mask[:, b]), -qh * TQ)
    idx = np.nonzero(~maskb)[0]
    nk = idx.shape[0]
    assert nk <= tk
    idxp = np.concatenate([idx, np.zeros(tk - nk, np.int64)])
    hbT = np.ascontiguousarray(hb.T).astype(f8)             # [D, T]
    m = {}
    # [128, DC*T] pre-arranged to the SBUF layout -> one full-rate DMA
    m["hTq"] = np.ascontiguousarray(
        hbT[:, :TQ].reshape(DC, 128, TQ).transpose(1, 0, 2)
           .reshape(128, DC * TQ))
    m["hTk"] = np.ascontiguousarray(
        hbT[:, idxp].reshape(DC, 128, tk).transpose(1, 0, 2)
           .reshape(128, DC * tk))
    m["hq"] = np.ascontiguousarray(hb[:TQ])                 # [TQ, D] f32
    # [D, F] -> [FC][128, DC*128]: block (p, dc) at [:, dc*128:...] holds
    # wT[dc*128:(dc+1)*128, p*128:(p+1)*128]
    wqT = (wq.T * WS).astype(f8)                            # [D, F]
    wkT = (wkv[:F].T * WS).astype(f8)
    wvT = (wkv[F:].T * WS).astype(f8)
    m["wqT"] = np.ascontiguousarray(
        wqT.reshape(DC, 128, FC, 128).transpose(2, 1, 0, 3)
           .reshape(FC, 128, DC * 128))
    m["wkT"] = np.ascontiguousarray(
        wkT.reshape(DC, 128, FC, 128).transpose(2, 1, 0, 3)
           .reshape(FC, 128, DC * 128))
    m["wvT"] = np.ascontiguousarray(
        wvT.reshape(DC, 128, NFS, 512).transpose(2, 1, 0, 3)
           .reshape(NFS, 128, DC * 512))
    m["woT"] = np.ascontiguousarray((wo.T * WS).astype(f8).reshape(FC, 128, D))
    mbias = np.full(tk, NEG_BIG, np.float32)
    mbias[:nk] = SHIFT
    m["maskbias"] = np.ascontiguousarray(mbias.reshape(KC, 128).T)
    if not simple_ln:
        m["g_rep"] = np.ascontiguousarray(
            np.broadcast_to(np.asarray(ln_g, np.float32), (128, D)))
        m["b_rep"] = np.ascontiguousarray(
            np.broadcast_to(np.asarray(ln_b, np.float32), (128, D)))
    return m

# ======================================================================
# Host-side runner: shard, compile (cached), execute on 8 cores, gather.
# ======================================================================
_NC_CACHE = {}
LAST_RESULT = None  # BassKernelResults of the most recent kernel() call


def _get_nc(T, TQ, TK, D, NH, DH, simple_ln, nfp):
    key = (T, TQ, TK, D, NH, DH, simple_ln, nfp)
    if key not in _NC_CACHE:
        _NC_CACHE[key] = build_nc(T, TQ, TK, D, NH, DH, n_cores=8,
                                  simple_ln=simple_ln, nfp=nfp, debug=False)
    return _NC_CACHE[key]


def kernel(h, attn_mask, wq, wkv, wo, ln_g, ln_b):
    """Full-input MultiHeadAttn forward on 8 NeuronCores.

    h: [T, B, D] f32; attn_mask: [T, B] bool (True = masked key);
    wq: [F, D]; wkv: [2F, D]; wo: [D, F]; ln_g/ln_b: [D].
    Returns [T, B, D] f32 = layer_norm(h + attn(h)).
    """
    from concourse.bass_utils import run_bass_kernel_spmd
    global LAST_RESULT

    h = np.asarray(h)
    attn_mask = np.asarray(attn_mask)
    wq = np.asarray(wq, np.float32)
    wkv = np.asarray(wkv, np.float32)
    wo = np.asarray(wo, np.float32)
    ln_g = np.asarray(ln_g, np.float32)
    ln_b = np.asarray(ln_b, np.float32)

    T, B, D = h.shape
    NH = 16
    DH = wq.shape[0] // NH
    assert 2 * B == 8, "sharding assumes batch 4 over 8 cores"
    TQ = T // 2
    TK = min(choose_tk(attn_mask), T)
    simple_ln = bool(np.all(ln_g == 1.0) and np.all(ln_b == 0.0))
    # j-tile pairs fully unmasked on every core (exp ops fuse over them):
    # per-batch unmasked counts are identical for both query-half cores.
    nk_min = int((~attn_mask).sum(axis=0).min())
    n_full = nk_min // 128          # leading fully-unmasked j-tiles
    nfp = max(0, min(TK // 256, n_full // 2))

    nc = _get_nc(T, TQ, TK, D, NH, DH, simple_ln, nfp)
    in_maps = [host_prep_core(c, TK, h, attn_mask, wq, wkv, wo, ln_g, ln_b,
                              NH=NH, DH=DH, simple_ln=simple_ln)
               for c in range(8)]
    res = run_bass_kernel_spmd(nc, in_maps, core_ids=list(range(8)))
    LAST_RESULT = res

    out = np.empty((T, B, D), np.float32)
    for c in range(8):
        b, qh = c // 2, c % 2
        out[qh * TQ:(qh + 1) * TQ, b, :] = res.results[c]["out"]
    return out


# revision 32
# speedup vs baseline: 1.0186x; 1.0126x over previous
"""Multi-head attention Bass/Tile kernel for TRN2, 8-core SPMD.

Sharding: core c handles batch b = c//2, query-half qh = c%2. The host
rotates the token axis per core so query rows sit at [0:TQ] (attention is
key-permutation invariant), and gathers the unmasked keys (mask compaction)
so K/V projection + attention only touch TK <= T key tokens.

v2: software-pipelined emission. Tile's scheduler is an out-of-order
per-engine list scheduler (priority = emission order), so projections are
emitted per head-pair interleaved with the attention loop: the ACT-bound
exp stream starts ~17us in (vs 103us when all projections precede it) and
later projections fill PE bubbles inside the attention phase.

Other changes vs v1:
  - softmax denominator: 1/den via ACT exp(-ln(den)) (both funcs live in
    the pinned natural_log_exp table set). Replaces a 53us DVE
    RECIPROCAL (iterative divide, ~6.5 cyc/elem) + two PSUM copies.
  - LayerNorm stats via DVE bn_stats/bn_aggr (one pass) instead of three
    ACT accumulation passes; fast path when ln_g==1 and ln_b==0.
  - wq/wk DMAd in [head-pair][D, 128] blocks (host pre-transposed) so
    K-proj for head-pair 0 only waits on ~5MB of input, not 8.4MB.

Matmul layouts (out = lhsT.T @ rhs, contraction on partitions):
  QT/KT [F, *] bf16 : lhsT=w*T [D,128] chunks, rhs=hT* [D,*] chunks
  V     [TK, F] bf16: lhsT=hTk chunk [D, t128], rhs=wvT [D, F-chunk]
  S^T   [j, (h0 i512 | h1 i512)] psum (2 banks): row-tiled head pair
  exp   one ACT op per j-tile: [128, 1024], bias=maskbias per-partition
  PV+den: lhsT=V[j,64]@(0,0)/(0,64) + ones[j,64], accum over j-tiles
  O     [t, D] psum : lhsT=AVT [f, t128], rhs=woT [f, D-chunk]
"""
import numpy as np
import ml_dtypes

import concourse.bass as bass
import concourse.tile as tile
from concourse import bacc, mybir

F32 = mybir.dt.float32
BF16 = mybir.dt.bfloat16
FP8 = mybir.dt.float8e4
DR = mybir.MatmulPerfMode.DoubleRow
AF = mybir.ActivationFunctionType
ALU = mybir.AluOpType

NEG_BIG = -1.0e30
# exp(s + SHIFT) keeps e comfortably inside fp8e4 range (max 448); the
# uniform scale cancels in the softmax quotient PV/den.
SHIFT = -4.0 * float(np.log(2.0))
# Weights are scaled x32 on the host so fp8e4 stores them as normals
# (raw values ~+-0.03 would land in the subnormal range). Compensated by
# the exp() scale immediate (scores) and the out-proj residual add.
WS = 32.0


def _pin_act_tables():
    """Force every ACT func we use (Exp, Ln) to resolve to the single
    `natural_log_exp_and_others` table set, so the kernel does exactly
    one ACT_TABLE_LOAD instead of thrashing (~2.6us per switch)."""
    import concourse.hw_specs as hw_specs
    if getattr(hw_specs, "_mha_tables_pinned", False):
        return
    orig = hw_specs.get_activation_tables

    def patched(module_arch):
        tabs = orig(module_arch)
        pin = "natural_log_exp_and_others"
        if pin in tabs:
            pinned_funcs = tabs[pin]
            for name, fns in tabs.items():
                if name != pin:
                    tabs[name] = fns - pinned_funcs
        return tabs

    hw_specs.get_activation_tables = patched
    import concourse.bacc as bacc_mod
    bacc_mod.get_activation_tables = patched
    hw_specs._mha_tables_pinned = True


def _chunks(total, step):
    out = []
    off = 0
    while off < total:
        out.append((off, min(step, total - off)))
        off += step
    return out


def build_nc(T, TQ, TK, D, NH, DH, n_cores=8, simple_ln=True, nfp=0,
             debug=False):
    """Build the single-core SPMD Bass program. TK = compacted key count.
    simple_ln: ln_g==1 and ln_b==0, skip the gamma/beta applies.
    nfp: number of j-tile PAIRS guaranteed fully unmasked on every core
    (their exp ops fuse into one N=2048 ACT op with a constant bias)."""
    F = NH * DH
    DC = D // 128        # D contraction chunks
    FC = F // 128        # feature chunks (2 heads per chunk, DH=64)
    KC = TK // 128       # key tiles
    TT = TQ // 128       # query t-tiles
    ICS = min(512, TQ)   # i-chunk size
    ICN = TQ // ICS
    DS = min(512, D)
    FS = 512             # V-proj f-chunk width
    NFS = F // FS
    assert DH == 64 and F % 128 == 0 and D % 128 == 0
    assert TQ % 128 == 0 and TK % 128 == 0 and ICN == 2
    assert DC % 2 == 0 and FC % 2 == 0 and KC >= 2

    SC = 1.0 / (float(np.sqrt(DH)) * WS * WS)   # exp scale immediate

    _pin_act_tables()
    nc = bacc.Bacc("TRN2", target_bir_lowering=False, debug=debug,
                   num_devices=n_cores)

    # ---- DRAM I/O ----
    # wq/wk host layout: [head-pair p][128 dpart, DC*128] so each
    # head-pair's weights arrive in one contiguous DMA. wv: [fc][128,
    # DC*512]. (Host does the block transposes.)
    hTq_d = nc.dram_tensor("hTq", [DC * 128, TQ], FP8, kind="ExternalInput")
    hTk_d = nc.dram_tensor("hTk", [DC * 128, TK], FP8, kind="ExternalInput")
    hq_d = nc.dram_tensor("hq", [TQ, D], F32, kind="ExternalInput")
    wqT_d = nc.dram_tensor("wqT", [FC, 128, DC * 128], FP8,
                           kind="ExternalInput")
    wkT_d = nc.dram_tensor("wkT", [FC, 128, DC * 128], FP8,
                           kind="ExternalInput")
    wvT_d = nc.dram_tensor("wvT", [NFS, 128, DC * FS], FP8,
                           kind="ExternalInput")
    woT_d = nc.dram_tensor("woT", [FC, 128, D], FP8, kind="ExternalInput")
    mb_d = nc.dram_tensor("maskbias", [128, KC], F32, kind="ExternalInput")
    if not simple_ln:
        g_d = nc.dram_tensor("g_rep", [128, D], F32, kind="ExternalInput")
        b_d = nc.dram_tensor("b_rep", [128, D], F32, kind="ExternalInput")
    out_d = nc.dram_tensor("out", [TQ, D], F32, kind="ExternalOutput")

    with tile.TileContext(nc) as tc:
        with (
            tc.tile_pool(name="hpool", bufs=1) as hpool,
            tc.tile_pool(name="wts", bufs=1) as wts,
            tc.tile_pool(name="acts", bufs=1) as acts,
            tc.tile_pool(name="small", bufs=1) as small,
            tc.tile_pool(name="exps", bufs=10) as expp,
            tc.tile_pool(name="eln", bufs=6) as eln,
            tc.tile_pool(name="epi", bufs=2) as epi,
            tc.tile_pool(name="latex", bufs=4) as latex,
            tc.tile_pool(name="psS", bufs=2, space="PSUM") as psS,
            tc.tile_pool(name="psPV", bufs=2, space="PSUM") as psPV,
            tc.tile_pool(name="psP", bufs=2, space="PSUM") as psP,
        ):
            # ---- persistent SBUF tiles ----
            hTq = hpool.tile([128, DC * TQ], FP8, tag="htq")
            hTk = hpool.tile([128, DC * TK], FP8, tag="htk")
            # wq/wk indexed [(p*DC + dc)*128 + col]; wv [(fc*DC + dc)*FS + c]
            wqT = wts.tile([128, FC * DC * 128], FP8, tag="wq")
            wkT = wts.tile([128, FC * DC * 128], FP8, tag="wk")
            wvT = wts.tile([128, NFS * DC * FS], FP8, tag="wv")
            QT = acts.tile([128, FC * TQ], BF16, tag="qt")
            KT = acts.tile([128, FC * TK], BF16, tag="kt")
            V = acts.tile([128, KC * F], BF16, tag="v")
            AVT = acts.tile([128, FC * TQ], FP8, tag="avt")
            ones = small.tile([128, 64], BF16, tag="ones")
            mb = small.tile([128, KC], F32, tag="mb")
            eps_t = small.tile([128, 1], F32, tag="eps")
            shift_t = small.tile([128, 1], F32, tag="shift")
            if not simple_ln:
                g_re = small.tile([128, D], F32, tag="g")
                b_re = small.tile([128, D], F32, tag="b")

            nc.vector.memset(ones[:], 1.0)
            nc.vector.memset(eps_t[:], 1e-5)
            nc.vector.memset(shift_t[:], SHIFT)
            nc.sync.dma_start(mb[:], mb_d[:])

            def dma_wq(p):
                if p < FC:
                    nc.sync.dma_start(
                        wqT[:, p * DC * 128:(p + 1) * DC * 128], wqT_d[p])

            def dma_wk(p):
                if p < FC:
                    nc.sync.dma_start(
                        wkT[:, p * DC * 128:(p + 1) * DC * 128], wkT_d[p])

            def dma_wv(fc):
                nc.sync.dma_start(
                    wvT[:, fc * DC * FS:(fc + 1) * DC * FS], wvT_d[fc])

            # 3D views for DoubleRow operand pairs (contraction = 128
            # partitions x 2 along the named free axis).
            hTq3 = hTq[:].rearrange("p (dc t) -> p dc t", dc=DC)
            hTk3 = hTk[:].rearrange("p (dc t) -> p dc t", dc=DC)
            wq3 = wqT[:].rearrange("p (c x) -> p c x", c=FC * DC)
            wk3 = wkT[:].rearrange("p (c x) -> p c x", c=FC * DC)
            wv3 = wvT[:].rearrange("p (c x) -> p c x", c=NFS * DC)
            AVT3 = AVT[:].rearrange("p (fc t) -> p fc t", fc=FC)

            # ---- projection chain emitters (fp8 DoubleRow, dc-pairs) ----
            def q_proj(p, chunks=None):
                if p >= FC:
                    return
                for t0, tn in (chunks or _chunks(TQ, 512)):
                    ps = psP.tile([128, 512], F32, tag="P")
                    for dh in range(DC // 2):
                        nc.tensor.matmul(
                            ps[:, 0:tn],
                            wq3[:, p * DC + 2 * dh: p * DC + 2 * dh + 2, :],
                            hTq3[:, 2 * dh: 2 * dh + 2, t0:t0 + tn],
                            start=(dh == 0), stop=(dh == DC // 2 - 1),
                            perf_mode=DR)
                    nc.vector.tensor_copy(
                        QT[:, p * TQ + t0: p * TQ + t0 + tn], ps[:, 0:tn])

            def k_proj(p, chunks=None):
                if p >= FC:
                    return
                for t0, tn in (chunks or _chunks(TK, 512)):
                    ps = psP.tile([128, 512], F32, tag="P")
                    for dh in range(DC // 2):
                        nc.tensor.matmul(
                            ps[:, 0:tn],
                            wk3[:, p * DC + 2 * dh: p * DC + 2 * dh + 2, :],
                            hTk3[:, 2 * dh: 2 * dh + 2, t0:t0 + tn],
                            start=(dh == 0), stop=(dh == DC // 2 - 1),
                            perf_mode=DR)
                    nc.vector.tensor_copy(
                        KT[:, p * TK + t0: p * TK + t0 + tn], ps[:, 0:tn])

            def v_proj(jc, fc):
                f0 = fc * FS
                ps = psP.tile([128, 512], F32, tag="P")
                for dh in range(DC // 2):
                    nc.tensor.matmul(
                        ps[:],
                        hTk3[:, 2 * dh: 2 * dh + 2,
                             jc * 128:(jc + 1) * 128],
                        wv3[:, fc * DC + 2 * dh: fc * DC + 2 * dh + 2, :],
                        start=(dh == 0), stop=(dh == DC // 2 - 1),
                        perf_mode=DR)
                nc.vector.tensor_copy(
                    V[:, jc * F + f0: jc * F + f0 + FS], ps[:])

            def score_pair(s, off, hp, jc, io):
                """Emit the row-tiled score matmul pair for (hp, jc) into
                s[:, off:off+2*ICS] = [h0-i | h1-i]."""
                nc.tensor.matmul(
                    s[:, off:off + ICS],
                    KT[0:64, hp * TK + jc * 128: hp * TK + (jc + 1) * 128],
                    QT[0:64, hp * TQ + io: hp * TQ + io + ICS],
                    start=True, stop=True, tile_position=(0, 0))
                nc.tensor.matmul(
                    s[:, off + ICS:off + 2 * ICS],
                    KT[64:128, hp * TK + jc * 128: hp * TK + (jc + 1) * 128],
                    QT[64:128, hp * TQ + io: hp * TQ + io + ICS],
                    start=True, stop=True, tile_position=(64, 0))

            def attention(ic, hp):
                with tc.high_priority(offset=500000):
                    _attention(ic, hp)

            def _attention(ic, hp):
                io = ic * ICS
                h0, h1 = 2 * hp, 2 * hp + 1
                pvP = psPV.tile([128, ICS], F32, tag="pv")
                pvD = psPV.tile([128, ICS], F32, tag="pv")
                for jc in range(KC):
                    s = psS.tile([128, 2 * ICS], F32, tag="S")
                    score_pair(s, 0, hp, jc, io)
                    e = expp.tile([128, 2 * ICS], BF16, tag="e")
                    nc.scalar.activation(e[:], s[:], AF.Exp,
                                         bias=mb[:, jc:jc + 1], scale=SC)
                    st, sp = (jc == 0), (jc == KC - 1)
                    nc.tensor.matmul(
                        pvP[0:64, :],
                        V[:, jc * F + h0 * DH: jc * F + (h0 + 1) * DH],
                        e[:, 0:ICS], start=st, stop=sp,
                        tile_position=(0, 0), skip_group_check=True)
                    nc.tensor.matmul(
                        pvP[64:128, :],
                        V[:, jc * F + h1 * DH: jc * F + (h1 + 1) * DH],
                        e[:, ICS:2 * ICS], start=st, stop=sp,
                        tile_position=(0, 64), skip_group_check=True)
                    nc.tensor.matmul(
                        pvD[0:64, :], ones[:, 0:64],
                        e[:, 0:ICS], start=st, stop=sp,
                        tile_position=(0, 0), skip_group_check=True)
                    nc.tensor.matmul(
                        pvD[64:128, :], ones[:, 0:64],
                        e[:, ICS:2 * ICS], start=st, stop=sp,
                        tile_position=(0, 64), skip_group_check=True)
                return_normalize(hp, io, pvP, pvD)

            def return_normalize(hp, io, pvP, pvD):
                # 1/den = exp(-ln(den)) on ACT (stays in the pinned table
                # set); then one DVE multiply straight out of PSUM.
                lden = eln.tile([128, ICS], F32, tag="r")
                nc.scalar.activation(lden[:], pvD[:], AF.Ln)
                rden = eln.tile([128, ICS], F32, tag="r")
                nc.scalar.activation(rden[:], lden[:], AF.Exp, scale=-1.0)
                nc.vector.tensor_tensor(
                    AVT[:, hp * TQ + io: hp * TQ + io + ICS],
                    pvP[:], rden[:], op=ALU.mult)

            def oln_head(tt, fh0, fh1, x=None, hqt=None, pool=None):
                """O-proj partial over fc pairs [fh0, fh1) + residual/accum
                into x. Returns (x, hqt)."""
                first = x is None
                if first:
                    pool = pool or epi
                    x = pool.tile([128, D], F32, tag="x")
                    hqt = pool.tile([128, D], F32, tag="hqt")
                    nc.sync.dma_start(hqt[:],
                                      hq_d[tt * 128:(tt + 1) * 128, :])
                for d0, dn in _chunks(D, DS):
                    ps = psP.tile([128, 512], F32, tag="P")
                    for fh in range(fh0, fh1):
                        nc.tensor.matmul(
                            ps[:, 0:dn],
                            AVT3[:, 2 * fh:2 * fh + 2,
                                 tt * 128:(tt + 1) * 128],
                            wo3[:, 2 * fh:2 * fh + 2, d0:d0 + dn],
                            start=(fh == fh0), stop=(fh == fh1 - 1),
                            perf_mode=DR)
                    nc.vector.scalar_tensor_tensor(
                        x[:, d0:d0 + dn], ps[:, 0:dn], 1.0 / (WS * WS),
                        (hqt if first else x)[:, d0:d0 + dn],
                        op0=ALU.mult, op1=ALU.add)
                return x, hqt

            def oln_ln(tt, x):
                # mean/var in one DVE pass (bn hardware limit: 512/group)
                stats = epi.tile([128, 12], F32, tag="st")
                nc.vector.bn_stats(stats[:, 0:6], x[:, 0:512])
                nc.vector.bn_stats(stats[:, 6:12], x[:, 512:1024])
                mv = epi.tile([128, 4], F32, tag="mv")
                nc.vector.bn_aggr(mv[:, 0:2], stats[:])
                # rstd = exp(-0.5*ln(var+eps)); Ln+Exp stay in-table-set
                lnv = mv[:, 2:3]
                nc.scalar.activation(lnv, mv[:, 1:2], AF.Ln, bias=eps_t[:])
                rstd = mv[:, 3:4]
                nc.scalar.activation(rstd, lnv, AF.Exp, scale=-0.5)
                # s2 = -mu*rstd;  x = x*rstd + s2  (per-partition affine)
                s2 = epi.tile([128, 1], F32, tag="s2")
                nc.vector.scalar_tensor_tensor(
                    s2[:], mv[:, 0:1], -1.0, rstd, op0=ALU.mult, op1=ALU.mult)
                nc.vector.tensor_scalar(x[:], x[:], rstd, s2[:],
                                        op0=ALU.mult, op1=ALU.add)
                if not simple_ln:
                    nc.vector.tensor_tensor(x[:], x[:], g_re[:], op=ALU.mult)
                    nc.vector.tensor_tensor(x[:], x[:], b_re[:], op=ALU.add)
                nc.sync.dma_start(out_d[tt * 128:(tt + 1) * 128, :], x[:])

            def oln_tile(tt):
                x, hqt = oln_head(tt, 0, FC // 4)
                x, _ = oln_head(tt, FC // 4, FC // 2, x, hqt)
                oln_ln(tt, x)

            # ---- DMA head-start + stage-1 projections for the pipeline ----
            dma_wk(0)
            for dc in range(DC):
                nc.sync.dma_start(hTk[:, dc * TK:(dc + 1) * TK],
                                  hTk_d[dc * 128:(dc + 1) * 128, :])
            dma_wq(0)
            for dc in range(DC):
                nc.sync.dma_start(hTq[:, dc * TQ:(dc + 1) * TQ],
                                  hTq_d[dc * 128:(dc + 1) * 128, :])
            dma_wk(1)
            dma_wq(1)
            dma_wv(0)
            if not simple_ln:
                nc.sync.dma_start(g_re[:], g_d[:])
                nc.sync.dma_start(b_re[:], b_d[:])

            # first chunks only: unblocks scores (hp0, ic0, jc0-3) ASAP
            k_proj(0, chunks=[(0, 512)])
            q_proj(0, chunks=[(0, 512)])
            k_proj(0, chunks=_chunks(TK, 512)[1:])
            q_proj(0, chunks=_chunks(TQ, 512)[1:])
            for jc in range(KC):
                v_proj(jc, 0)
            dma_wq(2)
            dma_wk(2)
            q_proj(1)
            k_proj(1)

            # ---- pipelined main loop ----
            # ic0: attention per head-pair; the next head-pairs' proj (and
            # V f-chunk 1, late DMAs) emitted behind it as PE filler.
            woT = wo3 = None
            for hp in range(FC):
                attention(0, hp)
                dma_wq(hp + 3)
                dma_wk(hp + 3)
                q_proj(hp + 2)
                k_proj(hp + 2)
                if hp == 1:
                    dma_wv(1)
                    for jc in range(KC):
                        v_proj(jc, 1)
                if hp == 5:
                    # woT reuses wq's SBUF slot (wq is dead after
                    # q_proj(7), emitted this iteration).
                    woT = wts.tile([128, FC * D], FP8, tag="wq")
                    wo3 = woT[:].rearrange("p (fc d) -> p fc d", fc=FC)
                    for fc_ in range(FC):
                        nc.sync.dma_start(woT[:, fc_ * D:(fc_ + 1) * D],
                                          woT_d[fc_])
            # ic1: attention with ic0's out-proj+LN tiles as filler.
            late = {}
            for hp in range(FC):
                attention(1, hp)
                if hp % 2 == 1:
                    oln_tile(hp // 2)
                if hp == FC - 3:
                    # O-proj of the tail tiles over all but the last
                    # fc-pair becomes filler; only 1/4 remains at the end
                    for tt in range(TT // 2, TT):
                        late[tt] = oln_head(tt, 0, FC // 2 - 1, pool=latex)
            for tt in range(TT // 2, TT):
                x, _ = oln_head(tt, FC // 2 - 1, FC // 2, *late[tt])
                oln_ln(tt, x)

    nc.compile()
    return nc


def choose_tk(attn_mask):
    """Compacted key count: max unmasked count over batches, ceil to 128."""
    m = np.asarray(attn_mask)
    counts = (~m).sum(axis=0)
    tk = int(((int(counts.max()) + 127) // 128) * 128)
    return max(tk, 128)


def host_prep_core(c, tk, h, attn_mask, wq, wkv, wo, ln_g, ln_b,
                   NH=16, DH=64, simple_ln=True):
    """Build the per-core input map (numpy) for core c."""
    T, B, D = h.shape
    F = NH * DH
    TQ = T // 2
    KC = tk // 128
    DC = D // 128
    FC = F // 128
    NFS = F // 512
    b, qh = c // 2, c % 2
    f8 = ml_dtypes.float8_e4m3
    hb = np.roll(np.asarray(h[:, b, :], dtype=np.float32), -qh * TQ, axis=0)
    maskb = np.roll(np.asarray(attn_mask[:, b]), -qh * TQ)
    idx = np.nonzero(~maskb)[0]
    nk = idx.shape[0]
    assert nk <= tk
    idxp = np.concatenate([idx, np.zeros(tk - nk, np.int64)])
    hbT = np.ascontiguousarray(hb.T).astype(f8)             # [D, T]
    m = {}
    m["hTq"] = np.ascontiguousarray(hbT[:, :TQ])
    m["hTk"] = np.ascontiguousarray(hbT[:, idxp])
    m["hq"] = np.ascontiguousarray(hb[:TQ])                 # [TQ, D] f32
    # [D, F] -> [FC][128, DC*128]: block (p, dc) at [:, dc*128:...] holds
    # wT[dc*128:(dc+1)*128, p*128:(p+1)*128]
    wqT = (wq.T * WS).astype(f8)                            # [D, F]
    wkT = (wkv[:F].T * WS).astype(f8)
    wvT = (wkv[F:].T * WS).astype(f8)
    m["wqT"] = np.ascontiguousarray(
        wqT.reshape(DC, 128, FC, 128).transpose(2, 1, 0, 3)
           .reshape(FC, 128, DC * 128))
    m["wkT"] = np.ascontiguousarray(
        wkT.reshape(DC, 128, FC, 128).transpose(2, 1, 0, 3)
           .reshape(FC, 128, DC * 128))
    m["wvT"] = np.ascontiguousarray(
        wvT.reshape(DC, 128, NFS, 512).transpose(2, 1, 0, 3)
           .reshape(NFS, 128, DC * 512))
    m["woT"] = np.ascontiguousarray((wo.T * WS).astype(f8).reshape(FC, 128, D))
    mbias = np.full(tk, NEG_BIG, np.float32)
    mbias[:nk] = SHIFT
    m["maskbias"] = np.ascontiguousarray(mbias.reshape(KC, 128).T)
    if not simple_ln:
        m["g_rep"] = np.ascontiguousarray(
            np.broadcast_to(np.asarray(ln_g, np.float32), (128, D)))
        m["b_rep"] = np.ascontiguousarray(
            np.broadcast_to(np.asarray(ln_b, np.float32), (128, D)))
    return m

# ======================================================================
# Host-side runner: shard, compile (cached), execute on 8 cores, gather.
# ======================================================================
_NC_CACHE = {}
LAST_RESULT = None  # BassKernelResults of the most recent kernel() call


def _get_nc(T, TQ, TK, D, NH, DH, simple_ln, nfp):
    key = (T, TQ, TK, D, NH, DH, simple_ln, nfp)
    if key not in _NC_CACHE:
        _NC_CACHE[key] = build_nc(T, TQ, TK, D, NH, DH, n_cores=8,
                                  simple_ln=simple_ln, nfp=nfp, debug=False)
    return _NC_CACHE[key]


def kernel(h, attn_mask, wq, wkv, wo, ln_g, ln_b):
    """Full-input MultiHeadAttn forward on 8 NeuronCores.

    h: [T, B, D] f32; attn_mask: [T, B] bool (True = masked key);
    wq: [F, D]; wkv: [2F, D]; wo: [D, F]; ln_g/ln_b: [D].
    Returns [T, B, D] f32 = layer_norm(h + attn(h)).
    """
    from concourse.bass_utils import run_bass_kernel_spmd
    global LAST_RESULT

    h = np.asarray(h)
    attn_mask = np.asarray(attn_mask)
    wq = np.asarray(wq, np.float32)
    wkv = np.asarray(wkv, np.float32)
    wo = np.asarray(wo, np.float32)
    ln_g = np.asarray(ln_g, np.float32)
    ln_b = np.asarray(ln_b, np.float32)

    T, B, D = h.shape
    NH = 16
    DH = wq.shape[0] // NH
    assert 2 * B == 8, "sharding assumes batch 4 over 8 cores"
    TQ = T // 2
    TK = min(choose_tk(attn_mask), T)
    simple_ln = bool(np.all(ln_g == 1.0) and np.all(ln_b == 0.0))
    # j-tile pairs fully unmasked on every core (exp ops fuse over them):
    # per-batch unmasked counts are identical for both query-half cores.
    nk_min = int((~attn_mask).sum(axis=0).min())
    n_full = nk_min // 128          # leading fully-unmasked j-tiles
    nfp = max(0, min(TK // 256, n_full // 2))

    nc = _get_nc(T, TQ, TK, D, NH, DH, simple_ln, nfp)
    in_maps = [host_prep_core(c, TK, h, attn_mask, wq, wkv, wo, ln_g, ln_b,
                              NH=NH, DH=DH, simple_ln=simple_ln)
               for c in range(8)]
    res = run_bass_kernel_spmd(nc, in_maps, core_ids=list(range(8)))
    LAST_RESULT = res

    out = np.empty((T, B, D), np.float32)
    for c in range(8):
        b, qh = c // 2, c % 2
        out[qh * TQ:(qh + 1) * TQ, b, :] = res.results[c]["out"]
    return out
